# revision 15
# baseline (speedup 1.0000x reference)
"""Segment mean-pool (BERT lattice embedding) Trainium2 Bass kernel.

Full-input contract: kernel(hidden[64,512,768] f32, word_ids[64,512] i32,
num_tokens=400) -> [64,400,768] f32.

Strategy: data-parallel over batch across 8 NeuronCores (8 samples each).
Per sample b the ragged segment mean  out[t] = mean_{s: wid[s]==t} hidden[s]
is computed as a matmul on the PE array:

    A_T[s, t] = (word_ids[b, s] == t)            one-hot, built on-device
    psum[t, :] = sum_j A_T[j-chunk].T @ [hidden[b, j-chunk] | 1]
    out[t, h] = psum[t, h] * 1/max(count[t], 1)

A ones-column appended to the moving operand makes count[t] land in the
last psum column of the same matmuls. All matmuls run in float32r
(FP22-truncated fp32): full speed at N>=256, ~6e-5 relative error, and no
dtype casts of the 100 MB activation tensor.
"""

import os
import numpy as np

B, S, H, T = 64, 512, 768, 400
N_CORES = 8
B_LOC = B // N_CORES  # samples per core
P = 128
J = S // P  # contraction chunks per sample
HP1 = H + 2  # hidden columns + two ones columns (fp32r needs even N chunks)
N0 = 512  # first h-chunk (one full psum bank)
N1 = HP1 - N0  # 258: remaining h cols + count cols
M_CHUNKS = [(0, 128), (128, 128), (256, 128), (384, T - 384)]  # (t0, mw)

_CACHED = {}


def build_program():
    """Build + compile the single-core Bass program (same NEFF on all cores)."""
    import concourse.bass as bass  # noqa: F401
    import concourse.mybir as mybir
    import concourse.tile as tile
    from concourse import bacc

    nc = bacc.Bacc(
        "TRN2",
        target_bir_lowering=False,
        debug=False,
        enable_asserts=False,
        num_devices=N_CORES,
    )
    f32 = mybir.dt.float32
    f32r = mybir.dt.float32r

    # float32r == fp32 bit layout; the PE truncates to FP22 on read. Declaring
    # the whole hidden path float32r satisfies walrus's fp32r-producer rule
    # without any casts or extra copies.
    hidden_t = nc.dram_tensor("hidden", [B_LOC, S, H], f32r, kind="ExternalInput").ap()
    # word_ids arrive host-prearranged as [P, B_LOC, J] fp32 (values < 400 are
    # exact): partition = s % 128, so each (b, j) column is the per-partition
    # scalar for s-chunk j. tensor_scalar(is_equal) requires fp32 operands.
    wid_t = nc.dram_tensor("word_ids_pbj", [P, B_LOC, J], f32, kind="ExternalInput").ap()
    # All-ones [P, J, 2] fp32r: DMA'd into the ones columns of each hid tile
    # (memset can't write float32r, and fp32r matmul moving chunks must have
    # an even column count — hence two ones columns).
    ones_t = nc.dram_tensor("ones_pj2", [P, J, 2], f32r, kind="ExternalInput").ap()
    out_t = nc.dram_tensor("out", [B_LOC, T, H], f32, kind="ExternalOutput").ap()

    with tile.TileContext(nc) as tc:
        with tc.tile_pool(name="const", bufs=1) as const_pool, \
             tc.tile_pool(name="io", bufs=2) as io_pool, \
             tc.tile_pool(name="outp", bufs=2) as out_pool, \
             tc.tile_pool(name="small", bufs=4) as small_pool, \
             tc.tile_pool(name="psum", bufs=2, space="PSUM") as psum_pool:

            iota_t = const_pool.tile([P, T], f32, name="iota_t")
            nc.gpsimd.iota(
                iota_t,
                pattern=[[1, T]],
                base=0,
                channel_multiplier=0,
                allow_small_or_imprecise_dtypes=True,
            )

            wid_sb = const_pool.tile([P, B_LOC, J], f32, name="wid_sb")
            nc.sync.dma_start(out=wid_sb, in_=wid_t)

            for b in range(B_LOC):
                hid = io_pool.tile([P, J, HP1], f32r, name="hid", tag="hid")
                nc.sync.dma_start(
                    out=hid[:, :, 0:H],
                    in_=hidden_t[b].rearrange("(j p) h -> p j h", p=P),
                )
                nc.sync.dma_start(out=hid[:, :, H:HP1], in_=ones_t)

                aT = io_pool.tile([P, J, T], f32r, name="aT", tag="aT")
                for j in range(J):
                    nc.vector.tensor_scalar(
                        aT[:, j, :],
                        iota_t,
                        wid_sb[:, b, j : j + 1],
                        None,
                        op0=mybir.AluOpType.is_equal,
                    )

                obig = out_pool.tile([P, 3, H], f32, name="obig", tag="obig")
                osm = out_pool.tile([P, H], f32, name="osm", tag="osm")

                for mi, (t0, mw) in enumerate(M_CHUNKS):
                    ps0 = psum_pool.tile([P, N0], f32, name="ps0", tag="ps0")
                    ps1 = psum_pool.tile([P, N1], f32, name="ps1", tag="ps1")
                    for j in range(J):
                        nc.tensor.matmul(
                            ps0[:mw],
                            aT[:, j, t0 : t0 + mw],
                            hid[:, j, 0:N0],
                            start=(j == 0),
                            stop=(j == J - 1),
                        )
                    for j in range(J):
                        nc.tensor.matmul(
                            ps1[:mw],
                            aT[:, j, t0 : t0 + mw],
                            hid[:, j, N0:HP1],
                            start=(j == 0),
                            stop=(j == J - 1),
                        )

                    cnt = small_pool.tile([P, 1], f32, name="cnt", tag="cnt")
                    rec = small_pool.tile([P, 1], f32, name="rec", tag="rec")
                    nc.vector.tensor_scalar_max(cnt[:mw], ps1[:mw, N1 - 2 : N1 - 1], 1.0)
                    nc.vector.reciprocal(rec[:mw], cnt[:mw])

                    dest = obig[:, mi, :] if mi < 3 else osm
                    # out = psum * (1/count): ACT takes the big chunk (scale is
                    # a per-partition AP), DVE the rest — balances engines.
                    nc.scalar.mul(dest[:mw, 0:N0], ps0[:mw], rec[:mw])
                    nc.vector.tensor_scalar_mul(
                        dest[:mw, N0:H], ps1[:mw, 0 : H - N0], rec[:mw]
                    )  # ps1 cols 256,257 are the count columns; not copied out

                nc.sync.dma_start(
                    out=out_t[b, 0:384].rearrange("(m p) h -> p m h", p=P),
                    in_=obig,
                )
                nc.sync.dma_start(out=out_t[b, 384:T], in_=osm[: T - 384])

    nc.compile()
    return nc


def _prep_in_maps(hidden, word_ids):
    hidden = np.ascontiguousarray(np.asarray(hidden), dtype=np.float32).reshape(B, S, H)
    wid = np.ascontiguousarray(np.asarray(word_ids), dtype=np.int32).reshape(B, S)
    in_maps = []
    for i in range(N_CORES):
        hs = np.ascontiguousarray(hidden[i * B_LOC : (i + 1) * B_LOC])
        ws = wid[i * B_LOC : (i + 1) * B_LOC]
        # [B_LOC, S] -> [P, B_LOC, J]: partition = s % 128, j = s // 128
        wpbj = np.ascontiguousarray(
            ws.reshape(B_LOC, J, P).transpose(2, 0, 1).astype(np.float32)
        )
        in_maps.append(
            {
                "hidden": hs,
                "word_ids_pbj": wpbj,
                "ones_pj2": np.ones((P, J, 2), np.float32),
            }
        )
    return in_maps


def run(hidden, word_ids, trace=False, **trace_kwargs):
    from concourse import bass_utils

    if "nc" not in _CACHED:
        _CACHED["nc"] = build_program()
    nc = _CACHED["nc"]
    in_maps = _prep_in_maps(hidden, word_ids)
    res = bass_utils.run_bass_kernel_spmd(
        nc, in_maps, core_ids=list(range(N_CORES)), trace=trace, **trace_kwargs
    )
    out = np.concatenate([res.results[i]["out"] for i in range(N_CORES)], axis=0)
    return out.astype(np.float32, copy=False), res


def kernel(hidden, word_ids, num_tokens=None, **_unused):
    out, _ = run(hidden, word_ids, trace=False)
    return out
